# revision 1
# baseline (speedup 1.0000x reference)
"""Tensor-parallel causal self-attention (GQA + RoPE) for 8 TRN2 NeuronCores.

Sharding: batch(2) x kv-head-groups(4). Core c handles batch c//4 and kv heads
{2*(c%4), 2*(c%4)+1} (= 8 q heads). Each core computes a partial output
y_c[t, e] (its heads' contribution through wo); host sums the 4 partials per
batch.

Hardcoded problem shape: B=2, S=2048, D=2048, H=32, KV=8, HD=64, fp32.

On-device layout (per core, all tensors "transposed": feature dims on
partitions, tokens on free dim):
  xT   [16dc, 128p, 4tt, 512j]  x^T tiles (dm on partitions)
  QT   [128(2 heads x 64), 4hp, 2048t]   q projection, rope'd in place
  KT   [128(2 kv x 64), 2048t], KTsh = halves swapped (for q-half alignment)
  VT   [128, 2048] -> PE-transposed into Vp [128k, 2kv, 16kc, 65] (col 64 = 1.0
       so the PV matmul also produces softmax denominators in psum row 64)
  scores are computed TRANSPOSED: st[k, q] = K @ Q^T so that softmax(P^T)
  chunks feed the PV matmul directly as the moving operand (no transposes).
  Sums land in row 64 of the PV psum tile; division by sums happens on the
  oT stationary before the wo matmul (reciprocal + indicator-matmul
  partition-replication).
RoPE uses a host-side deinterleave permutation of wq/wk rows so rotation
pairs are partition blocks [0:32]/[32:64] per head (block swap + 3 DVE TTs).
All matmuls run as float32r (FP22, full PE rate at moving free dim >= 256).
"""

import os
import numpy as np

DIM = 2048
NH = 32
NKV = 8
HD = 64
SEQ = 2048
B = 2
NCORES = 8

_CACHE = {}


def _deinterleave(w, nheads):
    # rows of w are (head, hd) with rope pairs interleaved (2j, 2j+1).
    # Reorder per head to [x0 block (32), x1 block (32)].
    w = w.reshape(nheads, 32, 2, DIM)
    return np.concatenate([w[:, :, 0, :], w[:, :, 1, :]], axis=1)  # [nh, 64, D]


def _host_prep(x, freqs_cos, freqs_sin, wq, wk, wv, wo):
    """Build the per-core DRAM input dicts."""
    x = np.asarray(x, np.float32)
    freqs_cos = np.asarray(freqs_cos, np.float32)
    freqs_sin = np.asarray(freqs_sin, np.float32)
    wq = np.asarray(wq, np.float32)
    wk = np.asarray(wk, np.float32)
    wv = np.asarray(wv, np.float32)
    wo = np.asarray(wo, np.float32)

    wq_d = _deinterleave(wq, NH)           # [32, 64, D]
    wk_d = _deinterleave(wk, NKV)          # [8, 64, D]
    wv_r = wv.reshape(NKV, HD, DIM)        # [8, 64, D] (not permuted)

    # rope tables [128, 2048]
    ct = freqs_cos.T[:32]                  # [32, S]
    st = freqs_sin.T[:32]
    cosT = np.tile(ct, (4, 1)).astype(np.float32)
    sinT = np.tile(np.concatenate([-st, st], axis=0), (2, 1)).astype(np.float32)

    ident = np.concatenate([np.eye(HD), np.eye(HD)], axis=0).astype(np.float32)  # [128, 64]
    # ind33[k, p]: row 0 -> even-head slot (partitions 0:64), row 32 -> odd
    ind2 = np.zeros((33, 128), np.float32)
    ind2[0, 0:64] = 1.0
    ind2[32, 64:128] = 1.0

    xT_b = []
    for b in range(B):
        xtb = np.ascontiguousarray(x[b].T)                      # [D, S]
        xt = xtb.reshape(16, 128, 4, 512).transpose(2, 0, 1, 3)  # [tt, dc, p, j]
        xT_b.append(np.ascontiguousarray(xt))

    in_maps = []
    for c in range(NCORES):
        b, g = c // 4, c % 4
        wq_c = wq_d[8 * g:8 * g + 8].reshape(512, DIM)           # [512, D]
        wq_t = np.ascontiguousarray(
            wq_c.T.reshape(16, 128, 512).transpose(1, 0, 2))     # [128p, 16dc, 512h]
        wkv_c = np.concatenate(
            [wk_d[2 * g:2 * g + 2].reshape(128, DIM),
             wv_r[2 * g:2 * g + 2].reshape(128, DIM)], axis=0)   # [256, D]
        wkv_t = np.ascontiguousarray(
            wkv_c.T.reshape(16, 128, 256).transpose(1, 0, 2))    # [128p, 16dc, 256]
        woc = np.ascontiguousarray(wo[:, 512 * g:512 * g + 512].T)  # [512a, 2048e]
        wo_t = np.ascontiguousarray(
            woc.reshape(4, 128, 2048).transpose(1, 0, 2))        # [128p, 4hc, 2048e]
        in_maps.append({
            "xT": xT_b[b],
            "wq": wq_t,
            "wkv": wkv_t,
            "wo": wo_t,
            "cosT": cosT,
            "sinT": sinT,
            "ident": ident,
            "ind2": ind2,
            "ones16": np.ones((128, 16), np.float32),
        })
    return in_maps


def _build_kernel(tc, nc, io, mybir):
    from contextlib import ExitStack
    import concourse.bass as bass

    fp = mybir.dt.float32
    fpr = mybir.dt.float32r
    Exp = mybir.ActivationFunctionType.Exp

    def asf(ap):
        # fp32 view of an fp32r tile (same bits)
        return ap.bitcast(fp)

    with ExitStack() as ctx:
        consts = ctx.enter_context(tc.tile_pool(name="consts", bufs=1))
        big = ctx.enter_context(tc.tile_pool(name="big", bufs=1))

        id_s = consts.tile([128, 64], fp)
        nc.sync.dma_start(id_s[:], io["ident"].ap())
        ind2_s = consts.tile([33, 128], fp)
        nc.sync.dma_start(ind2_s[:], io["ind2"].ap())

        QT = big.tile([128, 4, 2048], fpr)
        KT = big.tile([128, 2048], fpr)
        VT = big.tile([128, 2048], fp)
        KTsh = big.tile([128, 2048], fpr)
        Vp = big.tile([128, 2, 16, 65], fpr)
        oTu = big.tile([128, 4, 2048], fpr)

        xT = io["xT"].ap()

        # ---- phase 1: Q/K/V projections (contraction over d_model) ----
        with tc.tile_pool(name="wqkv", bufs=1) as wp, \
             tc.tile_pool(name="xc", bufs=4) as xcp, \
             tc.tile_pool(name="pj", bufs=1, space="PSUM") as pjp:
            wq_s = wp.tile([128, 16, 512], fpr)
            nc.sync.dma_start(wq_s[:], io["wq"].ap().bitcast(fpr))
            wkv_s = wp.tile([128, 16, 256], fpr)
            nc.sync.dma_start(wkv_s[:], io["wkv"].ap().bitcast(fpr))
            for tt in range(4):
                acc = [pjp.tile([128, 512], fp, name=f"acc{i}", tag=f"acc{i}")
                       for i in range(6)]
                for dc in range(16):
                    xc = xcp.tile([128, 512], fpr)
                    nc.sync.dma_start(xc[:], xT[tt, dc].bitcast(fpr))
                    mk = dict(start=(dc == 0), stop=(dc == 15),
                              skip_group_check=True)
                    for hc in range(4):
                        nc.tensor.matmul(
                            acc[hc][:], wq_s[:, dc, hc * 128:(hc + 1) * 128],
                            xc[:], **mk)
                    nc.tensor.matmul(acc[4][:], wkv_s[:, dc, 0:128],
                                     xc[:], **mk)
                    nc.tensor.matmul(acc[5][:], wkv_s[:, dc, 128:256],
                                     xc[:], **mk)
                ts = slice(tt * 512, (tt + 1) * 512)
                for hc in range(4):
                    nc.scalar.copy(QT[:, hc, ts], acc[hc][:])
                nc.scalar.copy(KT[:, ts], acc[4][:])
                nc.scalar.copy(VT[:, ts], acc[5][:])

        # ---- phase 2: rope + KT shifted copy + V transpose ----
        with tc.tile_pool(name="ropec", bufs=1) as rcp, \
             tc.tile_pool(name="rope", bufs=2) as rp:
            cos_s = rcp.tile([128, 2048], fp)
            nc.sync.dma_start(cos_s[:], io["cosT"].ap())
            sin_s = rcp.tile([128, 2048], fp)
            nc.sync.dma_start(sin_s[:], io["sinT"].ap())

            def rope_inplace(dst):
                # dst is an fp32r AP; compute in fp32 views, final add
                # writes the fp32r tile (producer dtype = fp32r).
                sw = rp.tile([128, 2048], fp, name="sw", tag="sw")
                for blk in range(4):
                    sb = (blk ^ 1) * 32
                    nc.vector.tensor_copy(sw[blk * 32:(blk + 1) * 32, :],
                                          asf(dst)[sb:sb + 32, :])
                t2 = rp.tile([128, 2048], fp, name="t2", tag="t2")
                nc.vector.tensor_mul(t2[:], sw[:], sin_s[:])
                t1 = rp.tile([128, 2048], fp, name="t1", tag="t1")
                nc.vector.tensor_mul(t1[:], asf(dst)[:], cos_s[:])
                nc.vector.tensor_add(dst[:], t1[:], t2[:])

            for hp in range(4):
                rope_inplace(QT[:, hp, :])
            rope_inplace(KT[:, :])
            nc.vector.tensor_copy(KTsh[64:128, :], KT[0:64, :])
            nc.vector.tensor_copy(KTsh[0:64, :], KT[64:128, :])

        with tc.tile_pool(name="vt", bufs=2, space="PSUM") as vtp:
            for kv in range(2):
                nc.sync.dma_start(Vp[:, kv, :, 64],
                                  io["ones16"].ap().bitcast(fpr))
                for kc in range(16):
                    tp = vtp.tile([128, 64], fp)
                    nc.tensor.transpose(
                        tp[:], VT[kv * 64:(kv + 1) * 64, kc * 128:(kc + 1) * 128],
                        id_s[kv * 64:(kv + 1) * 64, :])
                    nc.scalar.copy(Vp[:, kv, kc, 0:64], tp[:])

        # ---- phase 3: attention (scores transposed: st[k, q]) ----
        sums_pool = ctx.enter_context(tc.tile_pool(name="sums", bufs=1))
        Sg = sums_pool.tile([33, 16, 512], fp)
        Rg = sums_pool.tile([33, 16, 512], fp)
        nc.vector.memset(Sg[:], 1.0)
        with tc.tile_pool(name="st", bufs=2, space="PSUM") as stp, \
             tc.tile_pool(name="pv", bufs=2, space="PSUM") as pvp, \
             tc.tile_pool(name="pt", bufs=3) as ptp:
            for h in range(8):
                kv = h // 4
                hp = h // 2
                qb = 64 * (h % 2)
                KTx = KT if 64 * kv == qb else KTsh
                for qt in range(4):
                    q0 = qt * 512
                    po = pvp.tile([65, 512], fp)
                    nck = 4 * qt + 4
                    ng = nck // 2
                    for g in range(ng):
                        st = stp.tile([128, 1024], fp)
                        pt = ptp.tile([128, 1024], fpr)
                        for j in range(2):
                            kc = 2 * g + j
                            nc.tensor.matmul(
                                st[:, j * 512:(j + 1) * 512],
                                KTx[qb:qb + 64, kc * 128:(kc + 1) * 128],
                                QT[qb:qb + 64, hp, q0:q0 + 512],
                                start=True, stop=True, skip_group_check=True)
                        nc.scalar.activation(pt[:], st[:], Exp, scale=0.125)
                        if g >= ng - 2:
                            nc.gpsimd.affine_select(
                                out=pt[:], in_=pt[:], base=q0 - 256 * g,
                                channel_multiplier=-1,
                                pattern=[[-128, 2], [1, 512]],
                                compare_op=mybir.AluOpType.is_ge, fill=0.0)
                        for j in range(2):
                            kc = 2 * g + j
                            nc.tensor.matmul(
                                po[:], Vp[:, kv, kc, :],
                                pt[:, j * 512:(j + 1) * 512],
                                start=(kc == 0), stop=(kc == nck - 1),
                                skip_group_check=True)
                    nc.scalar.copy(oTu[qb:qb + 64, hp, q0:q0 + 512], po[0:64, :])
                    par = 32 * (h % 2)
                    nc.vector.tensor_copy(Sg[par:par + 1, hp * 4 + qt, :],
                                          po[64:65, :])

        # ---- phase 3b: normalize oTu by softmax sums ----
        nc.vector.reciprocal_approx_fast(Rg[:], Sg[:])
        with tc.tile_pool(name="rr", bufs=2, space="PSUM") as rrp:
            for hp in range(4):
                for qt in range(4):
                    rr = rrp.tile([128, 512], fp)
                    # plain fp32 matmul (tiny): fp32r would need a rounded
                    # producer for Rg
                    nc.tensor.matmul(rr[:], ind2_s[:],
                                     Rg[0:33, hp * 4 + qt, :],
                                     start=True, stop=True,
                                     skip_group_check=True)
                    qs = slice(qt * 512, (qt + 1) * 512)
                    nc.vector.tensor_mul(oTu[:, hp, qs], asf(oTu)[:, hp, qs],
                                         rr[:])

        # ---- phase 4: output projection ----
        yap = io["y"].ap()
        with tc.tile_pool(name="wop", bufs=1) as wop, \
             tc.tile_pool(name="yo", bufs=6, space="PSUM") as yop, \
             tc.tile_pool(name="ys", bufs=4) as ysp:
            wo_s = wop.tile([128, 4, 2048], fpr)
            nc.sync.dma_start(wo_s[:], io["wo"].ap().bitcast(fpr))
            for tcn in range(16):
                for et in range(4):
                    yo = yop.tile([128, 512], fp)
                    for hc in range(4):
                        nc.tensor.matmul(
                            yo[:], oTu[:, hc, tcn * 128:(tcn + 1) * 128],
                            wo_s[:, hc, et * 512:(et + 1) * 512],
                            start=(hc == 0), stop=(hc == 3),
                            skip_group_check=True)
                    ys = ysp.tile([128, 512], fp)
                    if (tcn + et) % 2 == 0:
                        nc.scalar.copy(ys[:], yo[:])
                    else:
                        nc.vector.tensor_copy(ys[:], yo[:])
                    nc.sync.dma_start(yap[tcn, et], ys[:])


def _get_program():
    if "nc" in _CACHE:
        return _CACHE["nc"]
    import concourse.tile as tile
    from concourse import bacc, mybir

    nc = bacc.Bacc("TRN2", target_bir_lowering=False, debug=False,
                   num_devices=NCORES)
    fp = mybir.dt.float32
    io = {
        "xT": nc.dram_tensor("xT", [4, 16, 128, 512], fp, kind="ExternalInput"),
        "wq": nc.dram_tensor("wq", [128, 16, 512], fp, kind="ExternalInput"),
        "wkv": nc.dram_tensor("wkv", [128, 16, 256], fp, kind="ExternalInput"),
        "wo": nc.dram_tensor("wo", [128, 4, 2048], fp, kind="ExternalInput"),
        "cosT": nc.dram_tensor("cosT", [128, 2048], fp, kind="ExternalInput"),
        "sinT": nc.dram_tensor("sinT", [128, 2048], fp, kind="ExternalInput"),
        "ident": nc.dram_tensor("ident", [128, 64], fp, kind="ExternalInput"),
        "ind2": nc.dram_tensor("ind2", [33, 128], fp, kind="ExternalInput"),
        "ones16": nc.dram_tensor("ones16", [128, 16], fp, kind="ExternalInput"),
        "y": nc.dram_tensor("y", [16, 4, 128, 512], fp, kind="ExternalOutput"),
    }
    with tile.TileContext(nc) as tc:
        _build_kernel(tc, nc, io, mybir)
    nc.compile()
    _CACHE["nc"] = nc
    return nc


def _run(inputs, trace=False):
    from concourse.bass_utils import run_bass_kernel_spmd

    nc = _get_program()
    in_maps = _host_prep(**inputs)
    res = run_bass_kernel_spmd(nc, in_maps, core_ids=list(range(NCORES)),
                               trace=trace)
    parts = [r_["y"].transpose(0, 2, 1, 3).reshape(SEQ, DIM)
             for r_ in res.results]
    out = np.stack([
        parts[0] + parts[1] + parts[2] + parts[3],
        parts[4] + parts[5] + parts[6] + parts[7],
    ]).astype(np.float32)
    return out, res


def kernel(**inputs):
    out, _ = _run(inputs, trace=False)
    return out

